# revision 40
# baseline (speedup 1.0000x reference)
"""DGCRN Trainium2 Bass kernel (v2 — fp8 DoubleRow + bf16 mega-tiles).

Problem: nn_DGCRN_67327907332247 (B=32, T=12, N=512, DIN=2, HID=64, CHEB_K=3,
EMB=10, DOUT=1, YCOV=1). Data-parallel over batch: 8 cores x 4 batches each.

Design notes (vs v1):
 - All state/weights bf16 (PE still 1 cycle/row; DVE gets 2x_1p on TT ops).
 - The dominant A-apply matmuls run fp8e4m3 in DoubleRow perf mode
   (2 k-subtiles per pass, 0.5 cycles/row => 4x vs f32r). Supports are
   stored as 128*A^T / 128*(2(A^2)^T) fp8 "mega" tiles [128, 4*512];
   pre-projections v=x*W as fp8 [128, 1024] per batch. The x128 scale
   keeps softmax entries in fp8-normal range; sigmoid/tanh read PSUM with
   scale=1/128.
 - Chebyshev -I term folded host-side: gw0 = 128*(W0 - W2) so the k=2
   support is just 2*A^2 (no diagonal fixup, no identity slabs).
 - Support build is transpose-free: S = ne@ne^T is symmetric, so with
   E = exp(relu(S)) (row blocks ex_i) and row sums D: A^T = E*D^{-1}
   (column scale via partition_broadcast of 1/rowsum), A = D^{-1}*E (row
   scale). (A^2)^T blocks come from fp8 DoubleRow matmuls of A/A^T blocks.
 - Batch lives in the free dim: states are [67, 4*512] mega tiles
   (rows: h 0:64, xt 64:66, const-ones 66 for the fused proj bias), so
   per-step x/y/out moves are single wide ops / single DMAs. Total DMA
   count ~50 (v1: 324 at ~1.7us serialized SP-queue time each).
 - proj: go = pw^T h' + pb via one matmul with lhsT=[pw;0;0;pb] against
   state rows 0:67 (ones row supplies the bias).
"""

import numpy as np

B = 32
NCORES = 8
BL = B // NCORES  # 4 local batches
T = 12
N = 512
NT = N // 128  # 4 node tiles
DIN = 2
HID = 64
EMB = 10
CIN = DIN + HID  # 66
SR = 67  # state rows: h(64) + xt(2) + ones(1)
K = 3
# Support scale: fp8e4 (IEEE e4m3) tops out at 240, so 64*A (<=64) and
# 2*64*A^2 (<=128) stay in range; quantization headroom ~2x.
SCALE = 64.0

_CACHE = {}


def _build_module():
    import concourse.bacc as bacc
    import concourse.mybir as mybir
    from concourse import masks, tile

    f32 = mybir.dt.float32
    bf16 = mybir.dt.bfloat16
    f8 = mybir.dt.float8e4
    Act = mybir.ActivationFunctionType
    DR = mybir.MatmulPerfMode.DoubleRow

    nc = bacc.Bacc("TRN2", target_bir_lowering=False, debug=False)

    x_d = nc.dram_tensor("x", [2 * T, BL * N], bf16, kind="ExternalInput").ap()
    y_d = nc.dram_tensor("y", [T, BL * N], bf16, kind="ExternalInput").ap()
    ne_d = nc.dram_tensor("ne", [128, NT * EMB], f32, kind="ExternalInput").ap()
    w_names = ["egws", "egw0", "euws", "euw0", "dgws", "dgw0", "duws", "duw0"]
    w_shapes = {
        "egws": [CIN, 4 * HID], "egw0": [CIN, 2 * HID],
        "euws": [CIN, 2 * HID], "euw0": [CIN, HID],
        "dgws": [CIN, 4 * HID], "dgw0": [CIN, 2 * HID],
        "duws": [CIN, 2 * HID], "duw0": [CIN, HID],
    }
    w_d = {
        n: nc.dram_tensor(n, w_shapes[n], bf16, kind="ExternalInput").ap()
        for n in w_names
    }
    pwb_d = nc.dram_tensor("pwb", [SR, 1], bf16, kind="ExternalInput").ap()
    hwt_d = nc.dram_tensor("hwt", [HID, EMB], bf16, kind="ExternalInput").ap()
    egb_d = nc.dram_tensor("egb", [2 * HID, 1], f32, kind="ExternalInput").ap()
    eub_d = nc.dram_tensor("eub", [HID, 1], f32, kind="ExternalInput").ap()
    dgb_d = nc.dram_tensor("dgb", [2 * HID, 1], f32, kind="ExternalInput").ap()
    dub_d = nc.dram_tensor("dub", [HID, 1], f32, kind="ExternalInput").ap()
    hb_d = nc.dram_tensor("hb", [EMB, 1], f32, kind="ExternalInput").ap()
    ones_d = nc.dram_tensor("ones", [1, BL * N], bf16, kind="ExternalInput").ap()
    out_d = nc.dram_tensor("out", [T, BL * N], bf16, kind="ExternalOutput").ap()

    BN = BL * N  # 2048

    def bsl(b):
        return slice(b * N, (b + 1) * N)

    with tile.TileContext(nc) as tc:
        with (
            tc.tile_pool(name="const", bufs=1) as cp,
            tc.tile_pool(name="state", bufs=1) as sp,
            tc.tile_pool(name="work", bufs=2) as wp,
            tc.tile_pool(name="psum", bufs=1, space="PSUM") as pp,
        ):
            ident = cp.tile([128, 128], f32)
            masks.make_identity(nc, ident[:])

            # ---- input / weight loads -----------------------------------
            ne_nm = cp.tile([128, NT * EMB], f32)
            nc.sync.dma_start(ne_nm[:], ne_d)
            w = {n: cp.tile(w_shapes[n], bf16, name=n) for n in w_names}
            for n in w_names:
                nc.sync.dma_start(w[n][:], w_d[n])
            pwb = cp.tile([SR, 1], bf16)
            nc.sync.dma_start(pwb[:], pwb_d)
            hwt = cp.tile([HID, EMB], bf16)
            nc.sync.dma_start(hwt[:], hwt_d)
            egb = cp.tile([2 * HID, 1], f32)
            nc.sync.dma_start(egb[:], egb_d)
            eub = cp.tile([HID, 1], f32)
            nc.sync.dma_start(eub[:], eub_d)
            dgb = cp.tile([2 * HID, 1], f32)
            nc.sync.dma_start(dgb[:], dgb_d)
            dub = cp.tile([HID, 1], f32)
            nc.sync.dma_start(dub[:], dub_d)
            hb = cp.tile([EMB, 1], f32)
            nc.sync.dma_start(hb[:], hb_d)

            # ---- persistent state ---------------------------------------
            pp_state = [sp.tile([SR, BN], bf16, name=f"st{p}") for p in range(2)]
            cand = sp.tile([SR, BN], bf16, name="cand")
            nc.vector.memset(pp_state[0][0:HID, :], 0.0)
            # const-ones row 66 (engine ops can't start at partition 66;
            # DMAs can) + x0 into cur rows 64:66
            for t_ in pp_state:
                nc.sync.dma_start(t_[SR - 1 : SR, :], ones_d)
            nc.sync.dma_start(pp_state[0][HID:CIN, :], x_d[0:2, :])

            vsb = [sp.tile([128, 8 * 128], f8, name=f"vsb{b}") for b in range(BL)]
            vcsb = [sp.tile([128, NT * 128], f8, name=f"vcsb{b}") for b in range(BL)]
            # encoder candidate-path v, pair-packed bf16 (both pairs in one
            # tile: pair p at cols p*1024, inner [j(4), s(2), batch(2), 64])
            vcpair = sp.tile([128, 2048], bf16, name="vcpair")

            # encoder supports: fp8 hi+lo residual pair (zr path) + bf16
            # copies (candidate path runs bf16 matmuls); the concentrated
            # encoder softmax is precision-critical (see module docstring).
            atm_eh = cp.tile([128, 2048], f8, name="atm_eh")
            atm_el = cp.tile([128, 2048], f8, name="atm_el")
            at2m_eh = cp.tile([128, 2048], f8, name="at2m_eh")
            at2m_el = cp.tile([128, 2048], f8, name="at2m_el")
            atb_e = cp.tile([128, 2048], bf16, name="atb_e")
            at2b_e = cp.tile([128, 2048], bf16, name="at2b_e")
            # decoder supports (per batch): single fp8 (near-uniform softmax,
            # insensitive)
            atm_d = [cp.tile([128, 2048], f8, name=f"atm_d{b}") for b in range(BL)]
            at2m_d = [cp.tile([128, 2048], f8, name=f"at2m_d{b}") for b in range(BL)]

            def psum(tag, bufs):
                return pp.tile([128, 512], f32, tag=tag, bufs=bufs, name=tag)

            def cpy(eng, dst, src):
                if eng == "v":
                    nc.vector.tensor_copy(dst, src)
                elif eng == "s":
                    nc.scalar.copy(dst, src)
                else:
                    nc.gpsimd.tensor_copy(dst, src)

            # ---- support build (transpose-free, fp8 output) --------------
            # builds: list of (emit_score(i), mode) with mode "enc" or
            # ("dec", atm, at2m)
            def build_supports(builds, mid=None):
                exs, es4s = [], []
                for bi, (escore, _) in enumerate(builds):
                    ex = wp.tile([128, 2048], f32, tag="ex", bufs=2, name=f"ex{bi}")
                    es4 = wp.tile([128, NT], f32, tag="es4", bufs=4, name=f"es{bi}")
                    for i in range(NT):
                        ps = escore(i)
                        nc.vector.tensor_scalar_max(ps[:], ps[:], 0.0)
                        nc.scalar.activation(
                            ex[:, i * 512 : (i + 1) * 512],
                            ps[:],
                            Act.Exp,
                            accum_out=es4[:, i : i + 1],
                        )
                    exs.append(ex)
                    es4s.append(es4)
                bcs = []
                for bi in range(len(builds)):
                    es4 = es4s[bi]
                    nc.vector.reciprocal(es4[:], es4[:])
                    nc.vector.tensor_scalar_mul(es4[:], es4[:], SCALE)
                    # per-column transposes so every [1,128] strip lands on
                    # partition 0 (engine ops may not start at partitions
                    # 1..31, and DVE in/out must share partitions)
                    tp = psum("hc", 2)
                    for i in range(NT):
                        nc.tensor.matmul(
                            tp[0:1, i * 128 : (i + 1) * 128],
                            es4[:, i : i + 1],
                            ident[:],
                            is_transpose=True,
                            skip_group_check=True,
                        )
                    rrow = wp.tile([1, 512], f32, tag="rr4", bufs=2, name=f"rr{bi}")
                    nc.vector.tensor_copy(rrow[:], tp[0:1, 0:512])
                    bc = wp.tile([128, 512], f32, tag="bc", bufs=2, name=f"bc{bi}")
                    for i in range(NT):
                        nc.gpsimd.partition_broadcast(
                            bc[:, i * 128 : (i + 1) * 128],
                            rrow[:, i * 128 : (i + 1) * 128],
                        )
                    bcs.append(bc)
                if mid is not None:
                    mid()
                anms = []
                sub = mybir.AluOpType.subtract
                mult = mybir.AluOpType.mult
                for bi, (_, mode) in enumerate(builds):
                    ex, es4, bc = exs[bi], es4s[bi], bcs[bi]
                    enc = mode == "enc"
                    anm = wp.tile(
                        [128, 2048], bf16 if enc else f8, tag="anm", bufs=2,
                        name=f"an{bi}",
                    )
                    for i in range(NT):
                        sl = slice(i * 512, (i + 1) * 512)
                        nc.vector.tensor_scalar_mul(
                            anm[:, sl], ex[:, sl], es4[:, i : i + 1]
                        )
                        if enc:
                            nc.gpsimd.tensor_mul(atb_e[:, sl], ex[:, sl], bc[:])
                        else:
                            nc.gpsimd.tensor_mul(mode[1][:, sl], ex[:, sl], bc[:])
                    if enc:
                        for i in range(NT):
                            sl = slice(i * 512, (i + 1) * 512)
                            nc.scalar.copy(atm_eh[:, sl], atb_e[:, sl])
                            nc.vector.scalar_tensor_tensor(
                                atm_el[:, sl], atb_e[:, sl], 1.0, atm_eh[:, sl],
                                mult, sub,
                            )
                    anms.append(anm)
                for bi, (_, mode) in enumerate(builds):
                    anm = anms[bi]
                    enc = mode == "enc"
                    for j in range(NT):
                        ps2 = psum("zr", 2)
                        if enc:
                            # square from bf16 A so at2 keeps full accuracy
                            for k in range(NT):
                                nc.tensor.matmul(
                                    ps2[:],
                                    anm[:].rearrange(
                                        "p (k q m) -> p k q m", k=4, q=4, m=128
                                    )[:, k, j, :],
                                    atb_e[:, k * 512 : (k + 1) * 512],
                                    start=(k == 0),
                                    stop=(k == NT - 1),
                                    skip_group_check=True,
                                )
                            sl = slice(j * 512, (j + 1) * 512)
                            nc.scalar.mul(at2b_e[:, sl], ps2[:], 2.0 / SCALE)
                            nc.scalar.copy(at2m_eh[:, sl], at2b_e[:, sl])
                            nc.vector.scalar_tensor_tensor(
                                at2m_el[:, sl], at2b_e[:, sl], 1.0,
                                at2m_eh[:, sl], mult, sub,
                            )
                        else:
                            atm, at2m = mode[1], mode[2]
                            for g in range(2):
                                nc.tensor.matmul(
                                    ps2[:],
                                    anm[:].rearrange(
                                        "p (k q m) -> p k q m", k=4, q=4, m=128
                                    )[:, 2 * g : 2 * g + 2, j, :],
                                    atm[:, g * 1024 : (g + 1) * 1024].rearrange(
                                        "p (k n) -> p k n", k=2
                                    ),
                                    start=(g == 0),
                                    stop=(g == 1),
                                    perf_mode=DR,
                                    skip_group_check=True,
                                )
                            nc.scalar.mul(
                                at2m[:, j * 512 : (j + 1) * 512], ps2[:],
                                2.0 / SCALE,
                            )

            # ---- cell emitters -------------------------------------------
            def emit_vb(b, cur, gws):
                # v = x @ [W1|W2]: per j, out [128, 256]; two psum halves
                for h in range(2):
                    ps = psum("vb", 2)
                    for jj in range(2):
                        j = 2 * h + jj
                        nc.tensor.matmul(
                            ps[:, jj * 256 : (jj + 1) * 256],
                            cur[0:CIN, b * N + j * 128 : b * N + (j + 1) * 128],
                            gws[:],
                            start=True,
                            stop=True,
                            skip_group_check=True,
                        )
                    # scatter [v1|v2] pairs into vsb layout
                    # [v1_0..v1_3 | v2_0..v2_3]. GPSIMD cannot read PSUM, so
                    # PSUM->SBUF copies alternate Act/DVE.
                    src = ps.rearrange("p (jj s c) -> p jj s c", jj=2, s=2, c=128)
                    dst = vsb[b][:].rearrange(
                        "p (s j c) -> p j s c", s=2, j=4, c=128
                    )[:, 2 * h : 2 * h + 2, :, :]
                    cpy("s" if h == 0 else "v", dst, src)

            def emit_zr(b, cur, gw0, atm, at2m, atml=None, at2ml=None):
                ps = psum("zr", 2)
                nc.tensor.matmul(
                    ps[:], gw0[:], cur[0:CIN, bsl(b)],
                    start=True, stop=False, skip_group_check=True,
                )
                terms = [(atm, 0), (at2m, 1)]
                if atml is not None:
                    terms += [(atml, 0), (at2ml, 1)]
                for ti, (am, s) in enumerate(terms):
                    for g in range(2):
                        nc.tensor.matmul(
                            ps[:],
                            vsb[b][
                                :, s * 512 + g * 256 : s * 512 + (g + 1) * 256
                            ].rearrange("p (k m) -> p k m", k=2),
                            am[:, g * 1024 : (g + 1) * 1024].rearrange(
                                "p (k n) -> p k n", k=2
                            ),
                            start=False,
                            stop=(ti == len(terms) - 1 and g == 1),
                            perf_mode=DR,
                            skip_group_check=True,
                        )
                return ps

            def emit_vcb(b, uws, eng, enc):
                ps = psum("vcb", 2)
                for j in range(NT):
                    nc.tensor.matmul(
                        ps[:, j * 128 : (j + 1) * 128],
                        cand[0:CIN, b * N + j * 128 : b * N + (j + 1) * 128],
                        uws[:],
                        start=True,
                        stop=True,
                        skip_group_check=True,
                    )
                if enc:
                    # pair-packed bf16: pair p at cols p*1024, batch slot b%2
                    p, x = b // 2, b % 2
                    src = ps.rearrange("p (j s c) -> p j s c", j=4, s=2, c=64)
                    dst = vcpair[:, p * 1024 : (p + 1) * 1024].rearrange(
                        "p (j s x c) -> p j s x c", j=4, s=2, x=2, c=64
                    )[:, :, :, x, :]
                    cpy(eng, dst, src)
                else:
                    cpy(eng, vcsb[b][:], ps[:])

            def emit_hc_enc(b, hcp, uw0):
                # bf16 candidate path, per batch (matmul dst must sit at
                # psum partition 0 per the s3d3 ISA check)
                p, x = b // 2, b % 2
                nc.tensor.matmul(
                    hcp[0:HID, :],
                    uw0[:],
                    cand[0:CIN, bsl(b)],
                    start=True,
                    stop=False,
                    skip_group_check=True,
                )
                for j in range(NT):
                    for s in range(2):
                        nc.tensor.matmul(
                            hcp[0:HID, :],
                            vcpair[
                                :,
                                p * 1024 + j * 256 + s * 128 + x * 64 :
                                p * 1024 + j * 256 + s * 128 + (x + 1) * 64,
                            ],
                            (atb_e if s == 0 else at2b_e)[
                                :, j * 512 : (j + 1) * 512
                            ],
                            start=False,
                            stop=(j == NT - 1 and s == 1),
                            skip_group_check=True,
                        )

            def emit_hc(b, hcp, uw0, atm, at2m):
                out = hcp[0:HID, :]
                nc.tensor.matmul(
                    out, uw0[:], cand[0:CIN, bsl(b)],
                    start=True, stop=False, skip_group_check=True,
                )
                for s in range(2):
                    for g in range(2):
                        nc.tensor.matmul(
                            out,
                            vcsb[b][:, g * 256 : (g + 1) * 256].rearrange(
                                "p (k s c) -> p k s c", k=2, s=2, c=64
                            )[:, :, s, :],
                            (atm if s == 0 else at2m)[
                                :, g * 1024 : (g + 1) * 1024
                            ].rearrange("p (k n) -> p k n", k=2),
                            start=False,
                            stop=(s == 1 and g == 1),
                            perf_mode=DR,
                            skip_group_check=True,
                        )

            # ---- pipelined recurrent loop (independent pair streams) ------
            # Batch pairs P0=(0,1), P1=(2,3) have no data dependencies on
            # each other, so they run as two software streams offset by half
            # a cell. Each stream's cell is 4 stages:
            #   A: gate matmuls + sigmoid + z*h     (needs vsb from prev D)
            #   B: candidate pre-projections (vcb)
            #   C: hc matmuls + tanh + state update
            #   D: [dec: proj+go]  + next cell's vbuilds + xt rows
            # Steady-state emission: A(t,P0) D(t-1,P1) B(t,P0) A(t,P1)
            #   C(t,P0) B(t,P1) D(t,P0) C(t,P1) -> t+1. The in-order PE
            #   queue then never parks on a latency chain: every matmul's
            #   producers were queued >= 2 stages earlier.
            def pipe_loop(dec, gws, gw0, uws, uw0, gb, ub, atms, at2ms):
                enc = not dec
                toff = 0 if enc else T

                def st(t):
                    return pp_state[(t + toff) % 2], pp_state[(t + toff + 1) % 2]

                def psl(p):
                    return slice(2 * p * N, (2 * p + 2) * N)

                vcb_eng = ("s", "v", "s", "v")
                # go copies cross partitions (psum row 0 -> state row 64):
                # only the Act engine may do that
                go_eng = ("s", "s", "s", "s")
                ctx = {}  # (t, p) -> dict of tiles

                def stage_A(t, p):
                    cur, nxt = st(t)
                    tag = ("d" if dec else "e") + f"{t}p{p}"
                    # z and r both live on partitions 0:64 (r via the Act
                    # engine's cross-partition read of psum rows 64:128) so
                    # every DVE op downstream is same-partition legal.
                    zsb = wp.tile([HID, 2 * N], bf16, tag=f"zsb{p}", bufs=2,
                                  name=f"z{tag}")
                    rsb = wp.tile([HID, 2 * N], bf16, tag=f"rsb{p}", bufs=2,
                                  name=f"r{tag}")
                    hct = wp.tile([HID, 2 * N], bf16, tag=f"hct{p}", bufs=2,
                                  name=f"hc{tag}")
                    dtt = wp.tile([HID, 2 * N], bf16, tag=f"dt{p}", bufs=2,
                                  name=f"dt{tag}")
                    ctx[t, p] = dict(z=zsb, r=rsb, hct=hct, dt=dtt)
                    # xt prefetch for t+1 into nxt (pair columns only)
                    if t + 1 < T:
                        if enc:
                            nc.sync.dma_start(
                                nxt[HID:CIN, psl(p)],
                                x_d[2 * (t + 1) : 2 * (t + 2), psl(p)],
                            )
                        else:
                            nc.sync.dma_start(
                                nxt[HID + 1 : CIN, psl(p)],
                                y_d[t + 1 : t + 2, psl(p)],
                            )
                    if enc:
                        nc.vector.tensor_copy(
                            cand[HID:CIN, psl(p)], cur[HID:CIN, psl(p)]
                        )
                    for x in range(2):
                        b = 2 * p + x
                        if enc:
                            zp = emit_zr(
                                b, cur, gw0, atm_eh, at2m_eh, atm_el, at2m_el
                            )
                        else:
                            zp = emit_zr(b, cur, gw0, atms[b], at2ms[b])
                        xs = slice(x * N, (x + 1) * N)
                        nc.scalar.activation(
                            zsb[:, xs], zp[0:HID, :], Act.Sigmoid,
                            bias=gb[0:HID, :], scale=1.0 / SCALE,
                        )
                        nc.scalar.activation(
                            rsb[:, xs], zp[HID:128, :], Act.Sigmoid,
                            bias=gb[HID : 2 * HID, :], scale=1.0 / SCALE,
                        )
                    nc.vector.tensor_mul(
                        cand[0:HID, psl(p)], zsb[:], cur[0:HID, psl(p)]
                    )

                def stage_B(t, p):
                    emit_vcb(2 * p, uws, vcb_eng[2 * p], enc)
                    emit_vcb(2 * p + 1, uws, vcb_eng[2 * p + 1], enc)

                def stage_C(t, p):
                    cur, nxt = st(t)
                    c = ctx[t, p]
                    rsb, hct, dtt = c["r"], c["hct"], c["dt"]
                    hcps = []
                    for x in range(2):
                        b = 2 * p + x
                        hcp = psum("hc", 2)
                        if enc:
                            emit_hc_enc(b, hcp, uw0)
                        else:
                            emit_hc(b, hcp, uw0, atms[b], at2ms[b])
                        hcps.append(hcp)
                    for x in range(2):
                        b = 2 * p + x
                        xs = slice(x * N, (x + 1) * N)
                        nc.scalar.activation(
                            hct[:, xs], hcps[x][0:HID, :],
                            Act.Tanh, bias=ub[:], scale=1.0 / SCALE,
                        )
                        nc.vector.tensor_sub(
                            dtt[:, xs], cur[0:HID, bsl(b)], hct[:, xs]
                        )
                        nc.vector.tensor_mul(
                            dtt[:, xs], rsb[:, xs], dtt[:, xs]
                        )
                        nc.vector.tensor_add(
                            nxt[0:HID, bsl(b)], hct[:, xs], dtt[:, xs]
                        )

                def stage_D(t, p):
                    cur, nxt = st(t)
                    if dec:
                        for x in range(2):
                            b = 2 * p + x
                            gop = psum("vcb", 2)
                            nc.tensor.matmul(
                                gop[0:1, :],
                                pwb[:],
                                nxt[0:SR, bsl(b)],
                                start=True,
                                stop=True,
                                skip_group_check=True,
                            )
                            cpy(
                                go_eng[b],
                                nxt[HID : HID + 1, bsl(b)],
                                gop[0:1, :],
                            )
                        nc.sync.dma_start(
                            out_d[t : t + 1, psl(p)], nxt[HID : HID + 1, psl(p)]
                        )
                    if t + 1 < T:
                        emit_vb(2 * p, nxt, gws)
                        emit_vb(2 * p + 1, nxt, gws)
                        if dec:
                            nc.vector.tensor_copy(
                                cand[HID:CIN, psl(p)], nxt[HID:CIN, psl(p)]
                            )

                # prologue (t=0: vsb pre-emitted by the support-build mid)
                stage_A(0, 0)
                stage_B(0, 0)
                stage_A(0, 1)
                stage_C(0, 0)
                stage_B(0, 1)
                stage_D(0, 0)
                stage_C(0, 1)
                for t in range(1, T):
                    stage_A(t, 0)
                    stage_D(t - 1, 1)
                    stage_B(t, 0)
                    stage_A(t, 1)
                    stage_C(t, 0)
                    stage_B(t, 1)
                    stage_D(t, 0)
                    stage_C(t, 1)
                stage_D(T - 1, 1)

            # ---- encoder support ------------------------------------------
            # neT via PE transposes of ne_nm blocks
            tpp = psum("hc", 2)
            for i in range(NT):
                nc.tensor.matmul(
                    tpp[0:EMB, i * 128 : (i + 1) * 128],
                    ne_nm[:, i * EMB : (i + 1) * EMB],
                    ident[:],
                    is_transpose=True,
                    skip_group_check=True,
                )
            neT = cp.tile([EMB, 512], bf16)
            nc.vector.tensor_copy(neT[:], tpp[0:EMB, :])

            def enc_score(i):
                ps = psum("vb", 2)
                nc.tensor.matmul(
                    ps[:], neT[:, i * 128 : (i + 1) * 128], neT[:],
                    start=True, stop=True, skip_group_check=True,
                )
                return ps

            def enc_mid():
                for b in range(BL):
                    emit_vb(b, pp_state[0], w["egws"])

            build_supports([(enc_score, "enc")], mid=enc_mid)

            # ---- encoder loop ---------------------------------------------
            pipe_loop(
                False, w["egws"], w["egw0"], w["euws"], w["euw0"], egb, eub,
                None, None,
            )

            # ---- decoder supports (hyper-network) -------------------------
            h_fin = pp_state[T % 2]
            nc.vector.memset(h_fin[HID : HID + 1, :], 0.0)  # go_0 = 0
            nc.sync.dma_start(h_fin[HID + 1 : CIN, :], y_d[0:1, :])
            nc.vector.tensor_copy(cand[HID:CIN, :], h_fin[HID:CIN, :])

            nebs = []
            for b in range(BL):
                psh = psum("vb", 2)
                nc.tensor.matmul(
                    psh[0:EMB, :], hwt[:], h_fin[0:HID, bsl(b)],
                    start=True, stop=True, skip_group_check=True,
                )
                neb = wp.tile([EMB, 512], bf16, tag="neb", bufs=4, name=f"neb{b}")
                nc.scalar.activation(neb[:], psh[0:EMB, :], Act.Identity, bias=hb[:])
                nebs.append(neb)

            def dec_score(b):
                def f(i):
                    ps = psum("vb", 2)
                    nc.tensor.matmul(
                        ps[:], nebs[b][:, i * 128 : (i + 1) * 128], nebs[b][:],
                        start=True, stop=True, skip_group_check=True,
                    )
                    return ps
                return f

            def dec_mid():
                for b in range(BL):
                    emit_vb(b, h_fin, w["dgws"])

            build_supports(
                [(dec_score(b), ("dec", atm_d[b], at2m_d[b])) for b in range(BL)],
                mid=dec_mid,
            )

            # ---- decoder loop ---------------------------------------------
            pipe_loop(
                True, w["dgws"], w["dgw0"], w["duws"], w["duw0"], dgb, dub,
                atm_d, at2m_d,
            )

    nc.compile()
    return nc


def _get_module():
    if "nc" not in _CACHE:
        _CACHE["nc"] = _build_module()
    return _CACHE["nc"]


def _in_maps(inputs):
    import ml_dtypes

    bf16 = ml_dtypes.bfloat16
    f32 = np.float32

    def perm(W):
        # per k-block reorder rows [xt; h] -> [h; xt]
        out = []
        for k in range(K):
            blk = np.asarray(W[k * CIN : (k + 1) * CIN], f32)
            out.append(np.concatenate([blk[DIN:], blk[:DIN]], axis=0))
        return out

    def prep_w(gW, uW):
        g = perm(gW)
        u = perm(uW)
        return {
            "gws": np.concatenate([g[1], g[2]], axis=1).astype(bf16),
            "gw0": (SCALE * (g[0] - g[2])).astype(bf16),
            "uws": np.concatenate([u[1], u[2]], axis=1).astype(bf16),
            "uw0": (SCALE * (u[0] - u[2])).astype(bf16),
        }

    e = prep_w(inputs["enc_gW"], inputs["enc_uW"])
    d = prep_w(inputs["dec_gW"], inputs["dec_uW"])
    pwb = np.zeros((SR, 1), f32)
    pwb[0:HID, 0] = np.asarray(inputs["proj_W"], f32)[:, 0]
    pwb[SR - 1, 0] = float(np.asarray(inputs["proj_b"], f32)[0])
    ne = np.asarray(inputs["node_emb"], f32)  # [512, 10]
    ne_nm = ne.reshape(NT, 128, EMB).transpose(1, 0, 2).reshape(128, NT * EMB)

    shared = {
        "egws": e["gws"], "egw0": e["gw0"], "euws": e["uws"], "euw0": e["uw0"],
        "dgws": d["gws"], "dgw0": d["gw0"], "duws": d["uws"], "duw0": d["uw0"],
        "pwb": pwb.astype(bf16),
        "hwt": np.asarray(inputs["hyper_W"], f32).astype(bf16),
        "ne": np.ascontiguousarray(ne_nm),
        "egb": np.asarray(inputs["enc_gb"], f32).reshape(-1, 1),
        "eub": np.asarray(inputs["enc_ub"], f32).reshape(-1, 1),
        "dgb": np.asarray(inputs["dec_gb"], f32).reshape(-1, 1),
        "dub": np.asarray(inputs["dec_ub"], f32).reshape(-1, 1),
        "hb": np.asarray(inputs["hyper_b"], f32).reshape(-1, 1),
        "ones": np.ones((1, BL * N), dtype=bf16),
    }
    x = np.asarray(inputs["x"], f32)  # [B, T, N, 2]
    y = np.asarray(inputs["y_cov"], f32)  # [B, T, N, 1]
    maps = []
    for c in range(NCORES):
        xc = x[c * BL : (c + 1) * BL]  # [BL, T, N, 2]
        yc = y[c * BL : (c + 1) * BL]
        m = dict(shared)
        # x rows: [t, c] pairs -> [2T, BL*N]
        m["x"] = np.ascontiguousarray(
            xc.transpose(1, 3, 0, 2).reshape(2 * T, BL * N).astype(bf16)
        )
        m["y"] = np.ascontiguousarray(
            yc.transpose(1, 3, 0, 2).reshape(T, BL * N).astype(bf16)
        )
        maps.append(m)
    return maps


def kernel(**inputs) -> np.ndarray:
    from concourse.bass_utils import run_bass_kernel_spmd

    nc = _get_module()
    maps = _in_maps(inputs)
    res = run_bass_kernel_spmd(nc, maps, list(range(NCORES)))
    outs = []
    for c in range(NCORES):
        o = np.asarray(res.results[c]["out"], dtype=np.float32)  # [T, BL*N]
        outs.append(o.reshape(T, BL, N).transpose(1, 0, 2)[..., None])
    return np.concatenate(outs, axis=0).astype(np.float32)


# revision 44
# speedup vs baseline: 1.0743x; 1.0743x over previous
"""DGCRN Trainium2 Bass kernel (v2 — fp8 DoubleRow + bf16 mega-tiles).

Problem: nn_DGCRN_67327907332247 (B=32, T=12, N=512, DIN=2, HID=64, CHEB_K=3,
EMB=10, DOUT=1, YCOV=1). Data-parallel over batch: 8 cores x 4 batches each.

Design notes (vs v1):
 - All state/weights bf16 (PE still 1 cycle/row; DVE gets 2x_1p on TT ops).
 - The dominant A-apply matmuls run fp8e4m3 in DoubleRow perf mode
   (2 k-subtiles per pass, 0.5 cycles/row => 4x vs f32r). Supports are
   stored as 128*A^T / 128*(2(A^2)^T) fp8 "mega" tiles [128, 4*512];
   pre-projections v=x*W as fp8 [128, 1024] per batch. The x128 scale
   keeps softmax entries in fp8-normal range; sigmoid/tanh read PSUM with
   scale=1/128.
 - Chebyshev -I term folded host-side: gw0 = 128*(W0 - W2) so the k=2
   support is just 2*A^2 (no diagonal fixup, no identity slabs).
 - Support build is transpose-free: S = ne@ne^T is symmetric, so with
   E = exp(relu(S)) (row blocks ex_i) and row sums D: A^T = E*D^{-1}
   (column scale via partition_broadcast of 1/rowsum), A = D^{-1}*E (row
   scale). (A^2)^T blocks come from fp8 DoubleRow matmuls of A/A^T blocks.
 - Batch lives in the free dim: states are [67, 4*512] mega tiles
   (rows: h 0:64, xt 64:66, const-ones 66 for the fused proj bias), so
   per-step x/y/out moves are single wide ops / single DMAs. Total DMA
   count ~50 (v1: 324 at ~1.7us serialized SP-queue time each).
 - proj: go = pw^T h' + pb via one matmul with lhsT=[pw;0;0;pb] against
   state rows 0:67 (ones row supplies the bias).
"""

import numpy as np

B = 32
NCORES = 8
BL = B // NCORES  # 4 local batches
T = 12
N = 512
NT = N // 128  # 4 node tiles
DIN = 2
HID = 64
EMB = 10
CIN = DIN + HID  # 66
SR = 67  # state rows: h(64) + xt(2) + ones(1)
K = 3
# Support scale: fp8e4 (IEEE e4m3) tops out at 240, so 64*A (<=64) and
# 2*64*A^2 (<=128) stay in range; quantization headroom ~2x.
SCALE = 64.0

_CACHE = {}


def _build_module():
    import concourse.bacc as bacc
    import concourse.mybir as mybir
    from concourse import masks, tile

    f32 = mybir.dt.float32
    bf16 = mybir.dt.bfloat16
    f8 = mybir.dt.float8e4
    Act = mybir.ActivationFunctionType
    DR = mybir.MatmulPerfMode.DoubleRow

    nc = bacc.Bacc("TRN2", target_bir_lowering=False, debug=False)

    x_d = nc.dram_tensor("x", [2 * T, BL * N], bf16, kind="ExternalInput").ap()
    y_d = nc.dram_tensor("y", [T, BL * N], bf16, kind="ExternalInput").ap()
    ne_d = nc.dram_tensor("ne", [128, NT * EMB], f32, kind="ExternalInput").ap()
    w_names = ["egws", "egw0", "euws", "euw0", "dgws", "dgw0", "duws", "duw0"]
    w_shapes = {
        "egws": [CIN, 4 * HID], "egw0": [CIN, 2 * HID],
        "euws": [CIN, 2 * HID], "euw0": [CIN, HID],
        "dgws": [CIN, 4 * HID], "dgw0": [CIN, 2 * HID],
        "duws": [CIN, 2 * HID], "duw0": [CIN, HID],
    }
    w_d = {
        n: nc.dram_tensor(n, w_shapes[n], bf16, kind="ExternalInput").ap()
        for n in w_names
    }
    pwb_d = nc.dram_tensor("pwb", [SR, 1], bf16, kind="ExternalInput").ap()
    hwt_d = nc.dram_tensor("hwt", [HID, EMB], bf16, kind="ExternalInput").ap()
    egb_d = nc.dram_tensor("egb", [2 * HID, 1], f32, kind="ExternalInput").ap()
    eub_d = nc.dram_tensor("eub", [HID, 1], f32, kind="ExternalInput").ap()
    dgb_d = nc.dram_tensor("dgb", [2 * HID, 1], f32, kind="ExternalInput").ap()
    dub_d = nc.dram_tensor("dub", [HID, 1], f32, kind="ExternalInput").ap()
    hb_d = nc.dram_tensor("hb", [EMB, 1], f32, kind="ExternalInput").ap()
    ones_d = nc.dram_tensor("ones", [1, BL * N], bf16, kind="ExternalInput").ap()
    out_d = nc.dram_tensor("out", [T, BL * N], bf16, kind="ExternalOutput").ap()

    BN = BL * N  # 2048

    def bsl(b):
        return slice(b * N, (b + 1) * N)

    with tile.TileContext(nc) as tc:
        with (
            tc.tile_pool(name="const", bufs=1) as cp,
            tc.tile_pool(name="state", bufs=1) as sp,
            tc.tile_pool(name="work", bufs=2) as wp,
            tc.tile_pool(name="psum", bufs=1, space="PSUM") as pp,
        ):
            ident = cp.tile([128, 128], f32)
            masks.make_identity(nc, ident[:])

            # ---- input / weight loads -----------------------------------
            ne_nm = cp.tile([128, NT * EMB], f32)
            nc.sync.dma_start(ne_nm[:], ne_d)
            w = {n: cp.tile(w_shapes[n], bf16, name=n) for n in w_names}
            for n in w_names:
                nc.sync.dma_start(w[n][:], w_d[n])
            pwb = cp.tile([SR, 1], bf16)
            nc.sync.dma_start(pwb[:], pwb_d)
            hwt = cp.tile([HID, EMB], bf16)
            nc.sync.dma_start(hwt[:], hwt_d)
            egb = cp.tile([2 * HID, 1], f32)
            nc.sync.dma_start(egb[:], egb_d)
            eub = cp.tile([HID, 1], f32)
            nc.sync.dma_start(eub[:], eub_d)
            dgb = cp.tile([2 * HID, 1], f32)
            nc.sync.dma_start(dgb[:], dgb_d)
            dub = cp.tile([HID, 1], f32)
            nc.sync.dma_start(dub[:], dub_d)
            hb = cp.tile([EMB, 1], f32)
            nc.sync.dma_start(hb[:], hb_d)

            # ---- persistent state ---------------------------------------
            pp_state = [sp.tile([SR, BN], bf16, name=f"st{p}") for p in range(2)]
            cand = sp.tile([SR, BN], bf16, name="cand")
            nc.vector.memset(pp_state[0][0:HID, :], 0.0)
            # const-ones row 66 (engine ops can't start at partition 66;
            # DMAs can) + x0 into cur rows 64:66
            for t_ in pp_state:
                nc.sync.dma_start(t_[SR - 1 : SR, :], ones_d)
            nc.sync.dma_start(pp_state[0][HID:CIN, :], x_d[0:2, :])

            vsb = [sp.tile([128, 8 * 128], f8, name=f"vsb{b}") for b in range(BL)]
            vcsb = [sp.tile([128, NT * 128], f8, name=f"vcsb{b}") for b in range(BL)]
            # encoder candidate-path v, pair-packed bf16 (both pairs in one
            # tile: pair p at cols p*1024, inner [j(4), s(2), batch(2), 64])
            vcpair = sp.tile([128, 2048], bf16, name="vcpair")

            # encoder supports: fp8 hi+lo residual pair (zr path) + bf16
            # copies (candidate path runs bf16 matmuls); the concentrated
            # encoder softmax is precision-critical (see module docstring).
            atm_eh = cp.tile([128, 2048], f8, name="atm_eh")
            atm_el = cp.tile([128, 2048], f8, name="atm_el")
            at2m_eh = cp.tile([128, 2048], f8, name="at2m_eh")
            at2m_el = cp.tile([128, 2048], f8, name="at2m_el")
            atb_e = cp.tile([128, 2048], bf16, name="atb_e")
            at2b_e = cp.tile([128, 2048], bf16, name="at2b_e")
            # decoder supports (per batch): single fp8 (near-uniform softmax,
            # insensitive)
            atm_d = [cp.tile([128, 2048], f8, name=f"atm_d{b}") for b in range(BL)]
            at2m_d = [cp.tile([128, 2048], f8, name=f"at2m_d{b}") for b in range(BL)]

            def psum(tag, bufs):
                return pp.tile([128, 512], f32, tag=tag, bufs=bufs, name=tag)

            def cpy(eng, dst, src):
                if eng == "v":
                    nc.vector.tensor_copy(dst, src)
                elif eng == "s":
                    nc.scalar.copy(dst, src)
                else:
                    nc.gpsimd.tensor_copy(dst, src)

            # ---- support build (transpose-free, fp8 output) --------------
            # builds: list of (emit_score(i), mode) with mode "enc" or
            # ("dec", atm, at2m)
            def build_supports(builds, mid=None):
                exs, es4s = [], []
                for bi, (escore, _) in enumerate(builds):
                    ex = wp.tile([128, 2048], f32, tag="ex", bufs=2, name=f"ex{bi}")
                    es4 = wp.tile([128, NT], f32, tag="es4", bufs=4, name=f"es{bi}")
                    for i in range(NT):
                        ps = escore(i)
                        nc.vector.tensor_scalar_max(ps[:], ps[:], 0.0)
                        nc.scalar.activation(
                            ex[:, i * 512 : (i + 1) * 512],
                            ps[:],
                            Act.Exp,
                            accum_out=es4[:, i : i + 1],
                        )
                    exs.append(ex)
                    es4s.append(es4)
                bcs = []
                for bi in range(len(builds)):
                    es4 = es4s[bi]
                    nc.vector.reciprocal(es4[:], es4[:])
                    nc.vector.tensor_scalar_mul(es4[:], es4[:], SCALE)
                    # per-column transposes so every [1,128] strip lands on
                    # partition 0 (engine ops may not start at partitions
                    # 1..31, and DVE in/out must share partitions)
                    tp = psum("hc", 2)
                    for i in range(NT):
                        nc.tensor.matmul(
                            tp[0:1, i * 128 : (i + 1) * 128],
                            es4[:, i : i + 1],
                            ident[:],
                            is_transpose=True,
                            skip_group_check=True,
                        )
                    rrow = wp.tile([1, 512], f32, tag="rr4", bufs=2, name=f"rr{bi}")
                    nc.vector.tensor_copy(rrow[:], tp[0:1, 0:512])
                    bc = wp.tile([128, 512], f32, tag="bc", bufs=2, name=f"bc{bi}")
                    for i in range(NT):
                        nc.gpsimd.partition_broadcast(
                            bc[:, i * 128 : (i + 1) * 128],
                            rrow[:, i * 128 : (i + 1) * 128],
                        )
                    bcs.append(bc)
                if mid is not None:
                    mid()
                anms = []
                sub = mybir.AluOpType.subtract
                mult = mybir.AluOpType.mult
                for bi, (_, mode) in enumerate(builds):
                    ex, es4, bc = exs[bi], es4s[bi], bcs[bi]
                    enc = mode == "enc"
                    anm = wp.tile(
                        [128, 2048], bf16 if enc else f8, tag="anm", bufs=2,
                        name=f"an{bi}",
                    )
                    for i in range(NT):
                        sl = slice(i * 512, (i + 1) * 512)
                        nc.vector.tensor_scalar_mul(
                            anm[:, sl], ex[:, sl], es4[:, i : i + 1]
                        )
                        if enc:
                            nc.gpsimd.tensor_mul(atb_e[:, sl], ex[:, sl], bc[:])
                        else:
                            nc.gpsimd.tensor_mul(mode[1][:, sl], ex[:, sl], bc[:])
                    if enc:
                        for i in range(NT):
                            sl = slice(i * 512, (i + 1) * 512)
                            nc.scalar.copy(atm_eh[:, sl], atb_e[:, sl])
                            nc.vector.scalar_tensor_tensor(
                                atm_el[:, sl], atb_e[:, sl], 1.0, atm_eh[:, sl],
                                mult, sub,
                            )
                    anms.append(anm)
                for bi, (_, mode) in enumerate(builds):
                    anm = anms[bi]
                    enc = mode == "enc"
                    for j in range(NT):
                        ps2 = psum("zr", 2)
                        if enc:
                            # square from bf16 A so at2 keeps full accuracy
                            for k in range(NT):
                                nc.tensor.matmul(
                                    ps2[:],
                                    anm[:].rearrange(
                                        "p (k q m) -> p k q m", k=4, q=4, m=128
                                    )[:, k, j, :],
                                    atb_e[:, k * 512 : (k + 1) * 512],
                                    start=(k == 0),
                                    stop=(k == NT - 1),
                                    skip_group_check=True,
                                )
                            sl = slice(j * 512, (j + 1) * 512)
                            nc.scalar.mul(at2b_e[:, sl], ps2[:], 2.0 / SCALE)
                            nc.scalar.copy(at2m_eh[:, sl], at2b_e[:, sl])
                            nc.vector.scalar_tensor_tensor(
                                at2m_el[:, sl], at2b_e[:, sl], 1.0,
                                at2m_eh[:, sl], mult, sub,
                            )
                        else:
                            atm, at2m = mode[1], mode[2]
                            for g in range(2):
                                nc.tensor.matmul(
                                    ps2[:],
                                    anm[:].rearrange(
                                        "p (k q m) -> p k q m", k=4, q=4, m=128
                                    )[:, 2 * g : 2 * g + 2, j, :],
                                    atm[:, g * 1024 : (g + 1) * 1024].rearrange(
                                        "p (k n) -> p k n", k=2
                                    ),
                                    start=(g == 0),
                                    stop=(g == 1),
                                    perf_mode=DR,
                                    skip_group_check=True,
                                )
                            nc.scalar.mul(
                                at2m[:, j * 512 : (j + 1) * 512], ps2[:],
                                2.0 / SCALE,
                            )

            # ---- cell emitters -------------------------------------------
            def emit_vb(b, cur, gws):
                # v = x @ [W1|W2]: per j, out [128, 256]; two psum halves
                for h in range(2):
                    ps = psum("vb", 2)
                    for jj in range(2):
                        j = 2 * h + jj
                        nc.tensor.matmul(
                            ps[:, jj * 256 : (jj + 1) * 256],
                            cur[0:CIN, b * N + j * 128 : b * N + (j + 1) * 128],
                            gws[:],
                            start=True,
                            stop=True,
                            skip_group_check=True,
                        )
                    # scatter [v1|v2] pairs into vsb layout
                    # [v1_0..v1_3 | v2_0..v2_3]. GPSIMD cannot read PSUM, so
                    # PSUM->SBUF copies alternate Act/DVE.
                    src = ps.rearrange("p (jj s c) -> p jj s c", jj=2, s=2, c=128)
                    dst = vsb[b][:].rearrange(
                        "p (s j c) -> p j s c", s=2, j=4, c=128
                    )[:, 2 * h : 2 * h + 2, :, :]
                    cpy("s" if (h == 0 and b % 2 == 0) else "v", dst, src)

            def emit_zr(b, cur, gw0, atm, at2m, atml=None, at2ml=None):
                ps = psum("zr", 2)
                nc.tensor.matmul(
                    ps[:], gw0[:], cur[0:CIN, bsl(b)],
                    start=True, stop=False, skip_group_check=True,
                )
                terms = [(atm, 0), (at2m, 1)]
                if atml is not None:
                    terms += [(atml, 0), (at2ml, 1)]
                for ti, (am, s) in enumerate(terms):
                    for g in range(2):
                        nc.tensor.matmul(
                            ps[:],
                            vsb[b][
                                :, s * 512 + g * 256 : s * 512 + (g + 1) * 256
                            ].rearrange("p (k m) -> p k m", k=2),
                            am[:, g * 1024 : (g + 1) * 1024].rearrange(
                                "p (k n) -> p k n", k=2
                            ),
                            start=False,
                            stop=(ti == len(terms) - 1 and g == 1),
                            perf_mode=DR,
                            skip_group_check=True,
                        )
                return ps

            def emit_vcb(b, uws, eng, enc):
                ps = psum("vcb", 2)
                for j in range(NT):
                    nc.tensor.matmul(
                        ps[:, j * 128 : (j + 1) * 128],
                        cand[0:CIN, b * N + j * 128 : b * N + (j + 1) * 128],
                        uws[:],
                        start=True,
                        stop=True,
                        skip_group_check=True,
                    )
                if enc:
                    # pair-packed bf16: pair p at cols p*1024, batch slot b%2
                    p, x = b // 2, b % 2
                    src = ps.rearrange("p (j s c) -> p j s c", j=4, s=2, c=64)
                    dst = vcpair[:, p * 1024 : (p + 1) * 1024].rearrange(
                        "p (j s x c) -> p j s x c", j=4, s=2, x=2, c=64
                    )[:, :, :, x, :]
                    cpy(eng, dst, src)
                else:
                    cpy(eng, vcsb[b][:], ps[:])

            def emit_hc_enc(b, hcp, uw0):
                # bf16 candidate path, per batch (matmul dst must sit at
                # psum partition 0 per the s3d3 ISA check)
                p, x = b // 2, b % 2
                nc.tensor.matmul(
                    hcp[0:HID, :],
                    uw0[:],
                    cand[0:CIN, bsl(b)],
                    start=True,
                    stop=False,
                    skip_group_check=True,
                )
                for j in range(NT):
                    for s in range(2):
                        nc.tensor.matmul(
                            hcp[0:HID, :],
                            vcpair[
                                :,
                                p * 1024 + j * 256 + s * 128 + x * 64 :
                                p * 1024 + j * 256 + s * 128 + (x + 1) * 64,
                            ],
                            (atb_e if s == 0 else at2b_e)[
                                :, j * 512 : (j + 1) * 512
                            ],
                            start=False,
                            stop=(j == NT - 1 and s == 1),
                            skip_group_check=True,
                        )

            def emit_hc(b, hcp, uw0, atm, at2m):
                out = hcp[0:HID, :]
                nc.tensor.matmul(
                    out, uw0[:], cand[0:CIN, bsl(b)],
                    start=True, stop=False, skip_group_check=True,
                )
                for s in range(2):
                    for g in range(2):
                        nc.tensor.matmul(
                            out,
                            vcsb[b][:, g * 256 : (g + 1) * 256].rearrange(
                                "p (k s c) -> p k s c", k=2, s=2, c=64
                            )[:, :, s, :],
                            (atm if s == 0 else at2m)[
                                :, g * 1024 : (g + 1) * 1024
                            ].rearrange("p (k n) -> p k n", k=2),
                            start=False,
                            stop=(s == 1 and g == 1),
                            perf_mode=DR,
                            skip_group_check=True,
                        )

            # ---- pipelined recurrent loop (independent pair streams) ------
            # Batch pairs P0=(0,1), P1=(2,3) have no data dependencies on
            # each other, so they run as two software streams offset by half
            # a cell. Each stream's cell is 4 stages:
            #   A: gate matmuls + sigmoid + z*h     (needs vsb from prev D)
            #   B: candidate pre-projections (vcb)
            #   C: hc matmuls + tanh + state update
            #   D: [dec: proj+go]  + next cell's vbuilds + xt rows
            # Steady-state emission: A(t,P0) D(t-1,P1) B(t,P0) A(t,P1)
            #   C(t,P0) B(t,P1) D(t,P0) C(t,P1) -> t+1. The in-order PE
            #   queue then never parks on a latency chain: every matmul's
            #   producers were queued >= 2 stages earlier.
            def pipe_loop(dec, gws, gw0, uws, uw0, gb, ub, atms, at2ms):
                enc = not dec
                toff = 0 if enc else T

                def st(t):
                    return pp_state[(t + toff) % 2], pp_state[(t + toff + 1) % 2]

                def psl(p):
                    return slice(2 * p * N, (2 * p + 2) * N)

                vcb_eng = ("s", "v", "s", "v")
                # go copies cross partitions (psum row 0 -> state row 64):
                # only the Act engine may do that
                go_eng = ("s", "s", "s", "s")
                ctx = {}  # (t, p) -> dict of tiles

                def stage_A(t, p):
                    cur, nxt = st(t)
                    tag = ("d" if dec else "e") + f"{t}p{p}"
                    # z and r both live on partitions 0:64 (r via the Act
                    # engine's cross-partition read of psum rows 64:128) so
                    # every DVE op downstream is same-partition legal.
                    zsb = wp.tile([HID, 2 * N], bf16, tag=f"zsb{p}", bufs=2,
                                  name=f"z{tag}")
                    rsb = wp.tile([HID, 2 * N], bf16, tag=f"rsb{p}", bufs=2,
                                  name=f"r{tag}")
                    hct = wp.tile([HID, 2 * N], bf16, tag=f"hct{p}", bufs=2,
                                  name=f"hc{tag}")
                    dtt = wp.tile([HID, 2 * N], bf16, tag=f"dt{p}", bufs=2,
                                  name=f"dt{tag}")
                    ctx[t, p] = dict(z=zsb, r=rsb, hct=hct, dt=dtt)
                    # xt prefetch for t+1 into nxt (pair columns only)
                    if t + 1 < T:
                        if enc:
                            nc.sync.dma_start(
                                nxt[HID:CIN, psl(p)],
                                x_d[2 * (t + 1) : 2 * (t + 2), psl(p)],
                            )
                        else:
                            nc.sync.dma_start(
                                nxt[HID + 1 : CIN, psl(p)],
                                y_d[t + 1 : t + 2, psl(p)],
                            )
                    if enc:
                        nc.vector.tensor_copy(
                            cand[HID:CIN, psl(p)], cur[HID:CIN, psl(p)]
                        )
                    for x in range(2):
                        b = 2 * p + x
                        if enc:
                            zp = emit_zr(
                                b, cur, gw0, atm_eh, at2m_eh, atm_el, at2m_el
                            )
                        else:
                            zp = emit_zr(b, cur, gw0, atms[b], at2ms[b])
                        xs = slice(x * N, (x + 1) * N)
                        nc.scalar.activation(
                            zsb[:, xs], zp[0:HID, :], Act.Sigmoid,
                            bias=gb[0:HID, :], scale=1.0 / SCALE,
                        )
                        nc.scalar.activation(
                            rsb[:, xs], zp[HID:128, :], Act.Sigmoid,
                            bias=gb[HID : 2 * HID, :], scale=1.0 / SCALE,
                        )
                    nc.vector.tensor_mul(
                        cand[0:HID, psl(p)], zsb[:], cur[0:HID, psl(p)]
                    )

                def stage_B(t, p):
                    emit_vcb(2 * p, uws, vcb_eng[2 * p], enc)
                    emit_vcb(2 * p + 1, uws, vcb_eng[2 * p + 1], enc)

                def stage_C(t, p):
                    cur, nxt = st(t)
                    c = ctx[t, p]
                    rsb, hct, dtt = c["r"], c["hct"], c["dt"]
                    hcps = []
                    for x in range(2):
                        b = 2 * p + x
                        hcp = psum("hc", 2)
                        if enc:
                            emit_hc_enc(b, hcp, uw0)
                        else:
                            emit_hc(b, hcp, uw0, atms[b], at2ms[b])
                        hcps.append(hcp)
                    for x in range(2):
                        b = 2 * p + x
                        xs = slice(x * N, (x + 1) * N)
                        nc.scalar.activation(
                            hct[:, xs], hcps[x][0:HID, :],
                            Act.Tanh, bias=ub[:], scale=1.0 / SCALE,
                        )
                        eng = nc.vector
                        eng.tensor_sub(
                            dtt[:, xs], cur[0:HID, bsl(b)], hct[:, xs]
                        )
                        eng.tensor_mul(
                            dtt[:, xs], rsb[:, xs], dtt[:, xs]
                        )
                        eng.tensor_add(
                            nxt[0:HID, bsl(b)], hct[:, xs], dtt[:, xs]
                        )

                def stage_D(t, p):
                    cur, nxt = st(t)
                    if dec:
                        for x in range(2):
                            b = 2 * p + x
                            gop = psum("vcb", 2)
                            nc.tensor.matmul(
                                gop[0:1, :],
                                pwb[:],
                                nxt[0:SR, bsl(b)],
                                start=True,
                                stop=True,
                                skip_group_check=True,
                            )
                            cpy(
                                go_eng[b],
                                nxt[HID : HID + 1, bsl(b)],
                                gop[0:1, :],
                            )
                        nc.sync.dma_start(
                            out_d[t : t + 1, psl(p)], nxt[HID : HID + 1, psl(p)]
                        )
                    if t + 1 < T:
                        emit_vb(2 * p, nxt, gws)
                        emit_vb(2 * p + 1, nxt, gws)
                        if dec:
                            nc.vector.tensor_copy(
                                cand[HID:CIN, psl(p)], nxt[HID:CIN, psl(p)]
                            )

                # prologue (t=0: vsb pre-emitted by the support-build mid)
                stage_A(0, 0)
                stage_B(0, 0)
                stage_A(0, 1)
                stage_C(0, 0)
                stage_B(0, 1)
                stage_D(0, 0)
                stage_C(0, 1)
                for t in range(1, T):
                    stage_A(t, 0)
                    stage_D(t - 1, 1)
                    stage_B(t, 0)
                    stage_A(t, 1)
                    stage_C(t, 0)
                    stage_B(t, 1)
                    stage_D(t, 0)
                    stage_C(t, 1)
                stage_D(T - 1, 1)

            # ---- encoder support ------------------------------------------
            # neT via PE transposes of ne_nm blocks
            tpp = psum("hc", 2)
            for i in range(NT):
                nc.tensor.matmul(
                    tpp[0:EMB, i * 128 : (i + 1) * 128],
                    ne_nm[:, i * EMB : (i + 1) * EMB],
                    ident[:],
                    is_transpose=True,
                    skip_group_check=True,
                )
            neT = cp.tile([EMB, 512], bf16)
            nc.vector.tensor_copy(neT[:], tpp[0:EMB, :])

            def enc_score(i):
                ps = psum("vb", 2)
                nc.tensor.matmul(
                    ps[:], neT[:, i * 128 : (i + 1) * 128], neT[:],
                    start=True, stop=True, skip_group_check=True,
                )
                return ps

            def enc_mid():
                for b in range(BL):
                    emit_vb(b, pp_state[0], w["egws"])

            build_supports([(enc_score, "enc")], mid=enc_mid)

            # ---- encoder loop ---------------------------------------------
            pipe_loop(
                False, w["egws"], w["egw0"], w["euws"], w["euw0"], egb, eub,
                None, None,
            )

            # ---- decoder supports (hyper-network) -------------------------
            h_fin = pp_state[T % 2]
            nc.vector.memset(h_fin[HID : HID + 1, :], 0.0)  # go_0 = 0
            nc.sync.dma_start(h_fin[HID + 1 : CIN, :], y_d[0:1, :])
            nc.vector.tensor_copy(cand[HID:CIN, :], h_fin[HID:CIN, :])

            nebs = []
            for b in range(BL):
                psh = psum("vb", 2)
                nc.tensor.matmul(
                    psh[0:EMB, :], hwt[:], h_fin[0:HID, bsl(b)],
                    start=True, stop=True, skip_group_check=True,
                )
                neb = wp.tile([EMB, 512], bf16, tag="neb", bufs=4, name=f"neb{b}")
                nc.scalar.activation(neb[:], psh[0:EMB, :], Act.Identity, bias=hb[:])
                nebs.append(neb)

            def dec_score(b):
                def f(i):
                    ps = psum("vb", 2)
                    nc.tensor.matmul(
                        ps[:], nebs[b][:, i * 128 : (i + 1) * 128], nebs[b][:],
                        start=True, stop=True, skip_group_check=True,
                    )
                    return ps
                return f

            def dec_mid():
                for b in range(BL):
                    emit_vb(b, h_fin, w["dgws"])

            build_supports(
                [(dec_score(b), ("dec", atm_d[b], at2m_d[b])) for b in range(BL)],
                mid=dec_mid,
            )

            # ---- decoder loop ---------------------------------------------
            pipe_loop(
                True, w["dgws"], w["dgw0"], w["duws"], w["duw0"], dgb, dub,
                atm_d, at2m_d,
            )

    nc.compile()
    return nc


def _get_module():
    if "nc" not in _CACHE:
        _CACHE["nc"] = _build_module()
    return _CACHE["nc"]


def _in_maps(inputs):
    import ml_dtypes

    bf16 = ml_dtypes.bfloat16
    f32 = np.float32

    def perm(W):
        # per k-block reorder rows [xt; h] -> [h; xt]
        out = []
        for k in range(K):
            blk = np.asarray(W[k * CIN : (k + 1) * CIN], f32)
            out.append(np.concatenate([blk[DIN:], blk[:DIN]], axis=0))
        return out

    def prep_w(gW, uW):
        g = perm(gW)
        u = perm(uW)
        return {
            "gws": np.concatenate([g[1], g[2]], axis=1).astype(bf16),
            "gw0": (SCALE * (g[0] - g[2])).astype(bf16),
            "uws": np.concatenate([u[1], u[2]], axis=1).astype(bf16),
            "uw0": (SCALE * (u[0] - u[2])).astype(bf16),
        }

    e = prep_w(inputs["enc_gW"], inputs["enc_uW"])
    d = prep_w(inputs["dec_gW"], inputs["dec_uW"])
    pwb = np.zeros((SR, 1), f32)
    pwb[0:HID, 0] = np.asarray(inputs["proj_W"], f32)[:, 0]
    pwb[SR - 1, 0] = float(np.asarray(inputs["proj_b"], f32)[0])
    ne = np.asarray(inputs["node_emb"], f32)  # [512, 10]
    ne_nm = ne.reshape(NT, 128, EMB).transpose(1, 0, 2).reshape(128, NT * EMB)

    shared = {
        "egws": e["gws"], "egw0": e["gw0"], "euws": e["uws"], "euw0": e["uw0"],
        "dgws": d["gws"], "dgw0": d["gw0"], "duws": d["uws"], "duw0": d["uw0"],
        "pwb": pwb.astype(bf16),
        "hwt": np.asarray(inputs["hyper_W"], f32).astype(bf16),
        "ne": np.ascontiguousarray(ne_nm),
        "egb": np.asarray(inputs["enc_gb"], f32).reshape(-1, 1),
        "eub": np.asarray(inputs["enc_ub"], f32).reshape(-1, 1),
        "dgb": np.asarray(inputs["dec_gb"], f32).reshape(-1, 1),
        "dub": np.asarray(inputs["dec_ub"], f32).reshape(-1, 1),
        "hb": np.asarray(inputs["hyper_b"], f32).reshape(-1, 1),
        "ones": np.ones((1, BL * N), dtype=bf16),
    }
    x = np.asarray(inputs["x"], f32)  # [B, T, N, 2]
    y = np.asarray(inputs["y_cov"], f32)  # [B, T, N, 1]
    maps = []
    for c in range(NCORES):
        xc = x[c * BL : (c + 1) * BL]  # [BL, T, N, 2]
        yc = y[c * BL : (c + 1) * BL]
        m = dict(shared)
        # x rows: [t, c] pairs -> [2T, BL*N]
        m["x"] = np.ascontiguousarray(
            xc.transpose(1, 3, 0, 2).reshape(2 * T, BL * N).astype(bf16)
        )
        m["y"] = np.ascontiguousarray(
            yc.transpose(1, 3, 0, 2).reshape(T, BL * N).astype(bf16)
        )
        maps.append(m)
    return maps


def kernel(**inputs) -> np.ndarray:
    from concourse.bass_utils import run_bass_kernel_spmd

    nc = _get_module()
    maps = _in_maps(inputs)
    res = run_bass_kernel_spmd(nc, maps, list(range(NCORES)))
    outs = []
    for c in range(NCORES):
        o = np.asarray(res.results[c]["out"], dtype=np.float32)  # [T, BL*N]
        outs.append(o.reshape(T, BL, N).transpose(1, 0, 2)[..., None])
    return np.concatenate(outs, axis=0).astype(np.float32)
